# revision 8
# baseline (speedup 1.0000x reference)
"""Trainium2 Bass kernel for BroadcastObstaclesToLanes (embedding lookup).

out[m, :] = obs_pos[same_obs_mask[m, 0], :]   m in [0, 16777216)

Sharding: M (lanes) split across 8 NeuronCores; the obs_pos table is
replicated so every core's gather is fully local.

Per core (2,097,152 tokens), two-stage gather:
  Stage 1 (GPSIMD dma_gather, custom SWDGE ucode): the table is viewed as
  32768 blocks of 32 rows (256B). Each token fetches the 256B block
  containing its row: block id q = idx >> 5 (int16), 8192 tokens per
  instruction, token i lands at dst[i % 128, i // 128, 0:64].
  Stage 2 (DVE): within-block select o = idx & 31 via
  mask = (o == iota_pair), masked = mask * block, pair-sum over the 32
  block rows -> [128, 64, 2] f32 exact result. 3 DVE ops per chunk,
  fully hidden under the gather.
Double-buffered across 256 chunks; sync engine streams idx chunks in and
results out.
"""

import numpy as np

N_OBS = 1048576
M_LANES = 16777216
NCORES = 8
MS = M_LANES // NCORES  # 2,097,152 tokens per core
P = 128
NIDX = 8192  # tokens per dma_gather
NG = MS // NIDX  # 256 gather chunks per core
C = NIDX // P  # 64 tokens per partition per chunk
NBLK = N_OBS // 32  # 32768 blocks of 32 rows (256B each)

_cached_nc = None


def _build():
    global _cached_nc
    if _cached_nc is not None:
        return _cached_nc

    import concourse.bacc as bacc
    import concourse.bass as bass
    from concourse import mybir
    from concourse.library_config import mlp

    nc = bacc.Bacc(
        "TRN2", target_bir_lowering=False, debug=False, num_devices=NCORES
    )
    tbl = nc.dram_tensor(
        "tbl", [NBLK, 64], mybir.dt.float32, kind="ExternalInput"
    )
    q16_d = nc.dram_tensor(
        "q16", [P, MS // 16], mybir.dt.int16, kind="ExternalInput"
    )
    o_d = nc.dram_tensor(
        "off", [P, NG * C], mybir.dt.float32, kind="ExternalInput"
    )
    iota_d = nc.dram_tensor(
        "iota", [P, 64], mybir.dt.float32, kind="ExternalInput"
    )
    out = nc.dram_tensor(
        "out", [NG, P, C, 2], mybir.dt.float32, kind="ExternalOutput"
    )

    W = NIDX // 16  # idx columns per chunk

    with (
        nc.Block() as block,
        nc.sbuf_tensor("dst0", [P, C, 64], mybir.dt.float32) as dst0,
        nc.sbuf_tensor("dst1", [P, C, 64], mybir.dt.float32) as dst1,
        nc.sbuf_tensor("msk", [P, C, 64], mybir.dt.float32) as msk,
        nc.sbuf_tensor("prod", [P, C, 64], mybir.dt.float32) as prod,
        nc.sbuf_tensor("red0", [P, C, 2], mybir.dt.float32) as red0,
        nc.sbuf_tensor("red1", [P, C, 2], mybir.dt.float32) as red1,
        nc.sbuf_tensor("idx0", [P, W], mybir.dt.int16) as idx0,
        nc.sbuf_tensor("idx1", [P, W], mybir.dt.int16) as idx1,
        nc.sbuf_tensor("o_sb", [P, NG * C], mybir.dt.float32) as o_sb,
        nc.sbuf_tensor("iota_sb", [P, 64], mybir.dt.float32) as iota_sb,
        nc.semaphore("s_pre") as s_pre,
        nc.semaphore("s_idx0") as s_idx0,
        nc.semaphore("s_idx1") as s_idx1,
        nc.semaphore("s_gat0") as s_gat0,
        nc.semaphore("s_gat1") as s_gat1,
        nc.semaphore("s_out0") as s_out0,
        nc.semaphore("s_out1") as s_out1,
        nc.semaphore("s_ext") as s_ext,
        nc.semaphore("s_dve") as s_dve,
    ):
        dsts = [dst0, dst1]
        reds = [red0, red1]
        idxs = [idx0, idx1]
        s_idx = [s_idx0, s_idx1]
        s_gat = [s_gat0, s_gat1]
        s_out = [s_out0, s_out1]

        @block.sync
        def _(sy: bass.BassEngine):
            sy.dma_start(o_sb[:], o_d.ap()[:]).then_inc(s_pre, 16)
            sy.dma_start(iota_sb[:], iota_d.ap()[:]).then_inc(s_pre, 16)
            for g in range(2):
                sy.dma_start(
                    idxs[g][:], q16_d.ap()[:, g * W : (g + 1) * W]
                ).then_inc(s_idx[g], 16)
            for g in range(NG):
                sy.wait_ge(s_ext, g + 1)
                sy.dma_start(out.ap()[g], reds[g % 2][:]).then_inc(
                    s_out[g % 2], 16
                )
                if g + 2 < NG:
                    sy.wait_ge(s_gat[g % 2], 16 * (g // 2 + 1))
                    sy.dma_start(
                        idxs[g % 2][:],
                        q16_d.ap()[:, (g + 2) * W : (g + 3) * W],
                    ).then_inc(s_idx[g % 2], 16)

        @block.gpsimd
        def _(gp: bass.BassGpSimd):
            gp.load_library(mlp)
            for g in range(NG):
                gp.wait_ge(s_idx[g % 2], 16 * (g // 2 + 1))
                if g >= 2:
                    gp.wait_ge(s_ext, g - 1)
                gp.dma_gather(
                    dsts[g % 2][:], tbl.ap()[:], idxs[g % 2][:],
                    NIDX, NIDX, 64, single_packet=False,
                ).then_inc(s_gat[g % 2], 16)

        @block.vector
        def _(ve: bass.BassEngine):
            ve.wait_ge(s_pre, 32)
            for g in range(NG):
                ve.wait_ge(s_gat[g % 2], 16 * (g // 2 + 1))
                if g >= 2:
                    ve.wait_ge(s_out[g % 2], 16 * (g // 2))
                o_slice = (
                    o_sb[:, g * C : (g + 1) * C]
                    .unsqueeze(2)
                    .broadcast_to([P, C, 64])
                )
                iota_b = iota_sb[:].unsqueeze(1).broadcast_to([P, C, 64])
                ve.tensor_tensor(
                    out=msk[:], in0=o_slice, in1=iota_b,
                    op=mybir.AluOpType.is_equal,
                ).then_inc(s_dve, 1)
                ve.wait_ge(s_dve, 2 * g + 1)
                ve.tensor_tensor(
                    out=prod[:], in0=msk[:], in1=dsts[g % 2][:],
                    op=mybir.AluOpType.mult,
                ).then_inc(s_dve, 1)
                ve.wait_ge(s_dve, 2 * g + 2)
                ve.tensor_reduce(
                    out=reds[g % 2][:],
                    in_=prod[:].rearrange("p c (w d) -> p c d w", w=32, d=2),
                    axis=mybir.AxisListType.X,
                    op=mybir.AluOpType.add,
                ).then_inc(s_ext, 1)

    nc.compile()
    _cached_nc = nc
    return nc


def _prepare_in_maps(obs_pos, same_obs_mask):
    tblblk = np.ascontiguousarray(
        np.asarray(obs_pos, dtype=np.float32).reshape(NBLK, 64)
    )
    idx32 = np.asarray(same_obs_mask).reshape(-1).astype(np.int32)
    iota = np.ascontiguousarray(
        np.tile((np.arange(64) // 2).astype(np.float32), (P, 1))
    )
    in_maps = []
    for c in range(NCORES):
        lanes = idx32[c * MS : (c + 1) * MS]
        q16 = (lanes >> 5).astype(np.int16)
        # wrap: token t at [t % 16, t // 16], replicated across 8 groups
        q16w = np.tile(np.ascontiguousarray(q16.reshape(MS // 16, 16).T), (8, 1))
        off = (
            (lanes & 31)
            .astype(np.float32)
            .reshape(NG, C, P)
            .transpose(2, 0, 1)
            .reshape(P, NG * C)
        )
        in_maps.append(
            {
                "tbl": tblblk,
                "q16": q16w,
                "off": np.ascontiguousarray(off),
                "iota": iota,
            }
        )
    return in_maps


def kernel(obs_pos, same_obs_mask):
    from concourse.bass_utils import run_bass_kernel_spmd

    nc = _build()
    in_maps = _prepare_in_maps(obs_pos, same_obs_mask)
    res = run_bass_kernel_spmd(nc, in_maps, core_ids=list(range(NCORES)))
    outs = []
    for r in res.results:
        o = r["out"]  # [NG, P, C, 2]; token t = g*8192 + c*128 + p
        outs.append(o.transpose(0, 2, 1, 3).reshape(MS, 2))
    return np.ascontiguousarray(np.concatenate(outs, axis=0))


# revision 11
# speedup vs baseline: 1.0092x; 1.0092x over previous
"""Trainium2 Bass kernel for BroadcastObstaclesToLanes (embedding lookup).

out[m, :] = obs_pos[same_obs_mask[m, 0], :]   m in [0, 16777216)

Sharding: M (lanes) split across 8 NeuronCores; the obs_pos table is
replicated so every core's gather is fully local.

Per core (2,097,152 tokens), two-stage gather:
  Stage 1 (GPSIMD dma_gather, custom SWDGE ucode): the table is viewed as
  32768 blocks of 32 rows (256B). Each token fetches the 256B block
  containing its row: block id q = idx >> 5 (int16), 8192 tokens per
  instruction, token i lands at dst[i % 128, i // 128, 0:64].
  Stage 2 (DVE): within-block select o = idx & 31 via
  mask = (o == iota_pair), masked = mask * block, pair-sum over the 32
  block rows -> [128, 64, 2] f32 exact result. 3 DVE ops per chunk,
  fully hidden under the gather.
Double-buffered across 256 chunks; sync engine streams idx chunks in and
results out.
"""

import numpy as np

N_OBS = 1048576
M_LANES = 16777216
NCORES = 8
MS = M_LANES // NCORES  # 2,097,152 tokens per core
P = 128
NIDX = 8192  # tokens per dma_gather
NG = MS // NIDX  # 256 gather chunks per core
C = NIDX // P  # 64 tokens per partition per chunk
NBLK = N_OBS // 32  # 32768 blocks of 32 rows (256B each)

_cached_nc = None


def _build():
    global _cached_nc
    if _cached_nc is not None:
        return _cached_nc

    import concourse.bacc as bacc
    import concourse.bass as bass
    from concourse import mybir
    from concourse.library_config import mlp

    nc = bacc.Bacc(
        "TRN2", target_bir_lowering=False, debug=False, num_devices=NCORES
    )
    tbl = nc.dram_tensor(
        "tbl", [NBLK, 64], mybir.dt.float32, kind="ExternalInput"
    )
    q16_d = nc.dram_tensor(
        "q16", [P, MS // 16], mybir.dt.int16, kind="ExternalInput"
    )
    o_d = nc.dram_tensor(
        "off", [P, NG * C], mybir.dt.float32, kind="ExternalInput"
    )
    iota_d = nc.dram_tensor(
        "iota", [P, 64], mybir.dt.float32, kind="ExternalInput"
    )
    out = nc.dram_tensor(
        "out", [NG, P, C, 2], mybir.dt.float32, kind="ExternalOutput"
    )

    W = NIDX // 16  # idx columns per chunk

    with (
        nc.Block() as block,
        nc.sbuf_tensor("dst0", [P, C, 64], mybir.dt.float32) as dst0,
        nc.sbuf_tensor("dst1", [P, C, 64], mybir.dt.float32) as dst1,
        nc.sbuf_tensor("msk", [P, C, 64], mybir.dt.float32) as msk,
        nc.sbuf_tensor("prod", [P, C, 64], mybir.dt.float32) as prod,
        nc.sbuf_tensor("red0", [P, C, 2], mybir.dt.float32) as red0,
        nc.sbuf_tensor("red1", [P, C, 2], mybir.dt.float32) as red1,
        nc.sbuf_tensor("idx0", [P, W], mybir.dt.int16) as idx0,
        nc.sbuf_tensor("idx1", [P, W], mybir.dt.int16) as idx1,
        nc.sbuf_tensor("o_sb", [P, NG * C], mybir.dt.float32) as o_sb,
        nc.sbuf_tensor("iota_sb", [P, 64], mybir.dt.float32) as iota_sb,
        nc.semaphore("s_pre") as s_pre,
        nc.semaphore("s_idx0") as s_idx0,
        nc.semaphore("s_idx1") as s_idx1,
        nc.semaphore("s_gat0") as s_gat0,
        nc.semaphore("s_gat1") as s_gat1,
        nc.semaphore("s_out0") as s_out0,
        nc.semaphore("s_out1") as s_out1,
        nc.semaphore("s_ext") as s_ext,
        nc.semaphore("s_dve") as s_dve,
    ):
        dsts = [dst0, dst1]
        reds = [red0, red1]
        idxs = [idx0, idx1]
        s_idx = [s_idx0, s_idx1]
        s_gat = [s_gat0, s_gat1]
        s_out = [s_out0, s_out1]

        @block.sync
        def _(sy: bass.BassEngine):
            sy.dma_start(o_sb[:], o_d.ap()[:]).then_inc(s_pre, 16)
            sy.dma_start(iota_sb[:], iota_d.ap()[:]).then_inc(s_pre, 16)
            for g in range(2):
                sy.dma_start(
                    idxs[g][:], q16_d.ap()[:, g * W : (g + 1) * W]
                ).then_inc(s_idx[g], 16)
            for g in range(NG - 2):
                sy.wait_ge(s_gat[g % 2], 16 * (g // 2 + 1))
                sy.dma_start(
                    idxs[g % 2][:],
                    q16_d.ap()[:, (g + 2) * W : (g + 3) * W],
                ).then_inc(s_idx[g % 2], 16)

        @block.scalar
        def _(sc: bass.BassEngine):
            for g in range(NG):
                sc.wait_ge(s_ext, g + 1)
                sc.dma_start(out.ap()[g], reds[g % 2][:]).then_inc(
                    s_out[g % 2], 16
                )

        @block.gpsimd
        def _(gp: bass.BassGpSimd):
            gp.load_library(mlp)
            for g in range(NG):
                gp.wait_ge(s_idx[g % 2], 16 * (g // 2 + 1))
                if g >= 2:
                    gp.wait_ge(s_ext, g - 1)
                gp.dma_gather(
                    dsts[g % 2][:], tbl.ap()[:], idxs[g % 2][:],
                    NIDX, NIDX, 64, single_packet=False,
                ).then_inc(s_gat[g % 2], 16)

        @block.vector
        def _(ve: bass.BassEngine):
            ve.wait_ge(s_pre, 32)
            for g in range(NG):
                ve.wait_ge(s_gat[g % 2], 16 * (g // 2 + 1))
                if g >= 2:
                    ve.wait_ge(s_out[g % 2], 16 * (g // 2))
                o_slice = (
                    o_sb[:, g * C : (g + 1) * C]
                    .unsqueeze(2)
                    .broadcast_to([P, C, 64])
                )
                iota_b = iota_sb[:].unsqueeze(1).broadcast_to([P, C, 64])
                ve.tensor_tensor(
                    out=msk[:], in0=o_slice, in1=iota_b,
                    op=mybir.AluOpType.is_equal,
                ).then_inc(s_dve, 1)
                ve.wait_ge(s_dve, 2 * g + 1)
                ve.tensor_tensor(
                    out=prod[:], in0=msk[:], in1=dsts[g % 2][:],
                    op=mybir.AluOpType.mult,
                ).then_inc(s_dve, 1)
                ve.wait_ge(s_dve, 2 * g + 2)
                ve.tensor_reduce(
                    out=reds[g % 2][:],
                    in_=prod[:].rearrange("p c (w d) -> p c d w", w=32, d=2),
                    axis=mybir.AxisListType.X,
                    op=mybir.AluOpType.add,
                ).then_inc(s_ext, 1)

    nc.compile()
    _cached_nc = nc
    return nc


def _prepare_in_maps(obs_pos, same_obs_mask):
    tblblk = np.ascontiguousarray(
        np.asarray(obs_pos, dtype=np.float32).reshape(NBLK, 64)
    )
    idx32 = np.asarray(same_obs_mask).reshape(-1).astype(np.int32)
    iota = np.ascontiguousarray(
        np.tile((np.arange(64) // 2).astype(np.float32), (P, 1))
    )
    in_maps = []
    for c in range(NCORES):
        lanes = idx32[c * MS : (c + 1) * MS]
        q16 = (lanes >> 5).astype(np.int16)
        # wrap: token t at [t % 16, t // 16], replicated across 8 groups
        q16w = np.tile(np.ascontiguousarray(q16.reshape(MS // 16, 16).T), (8, 1))
        off = (
            (lanes & 31)
            .astype(np.float32)
            .reshape(NG, C, P)
            .transpose(2, 0, 1)
            .reshape(P, NG * C)
        )
        in_maps.append(
            {
                "tbl": tblblk,
                "q16": q16w,
                "off": np.ascontiguousarray(off),
                "iota": iota,
            }
        )
    return in_maps


def kernel(obs_pos, same_obs_mask):
    from concourse.bass_utils import run_bass_kernel_spmd

    nc = _build()
    in_maps = _prepare_in_maps(obs_pos, same_obs_mask)
    res = run_bass_kernel_spmd(nc, in_maps, core_ids=list(range(NCORES)))
    outs = []
    for r in res.results:
        o = r["out"]  # [NG, P, C, 2]; token t = g*8192 + c*128 + p
        outs.append(o.transpose(0, 2, 1, 3).reshape(MS, 2))
    return np.ascontiguousarray(np.concatenate(outs, axis=0))
